# revision 1
# baseline (speedup 1.0000x reference)
"""Transformer block on 8 Trainium2 NeuronCores — head-sharded fp8 version.

Sharding: batch (4) x head-half (2). Core c owns batch b = c//2 and heads
[hg*6, hg*6+6), hg = c%2, over the FULL 1024-token sequence. Each core runs
LN1 + its 6 heads' QKV/attention and the row-shard of the output projection
(rows for its heads), producing a partial attention output for all 1024
tokens. The pair's partials are summed with a per-half ReduceScatter
(groups [[0,1],[2,3],[4,5],[6,7]]), leaving each core 2x256 own tokens;
each core then adds its residual slice (x + bo, prepped host-side), runs
LN2 + FFN on its 512 tokens and writes y.

Dtypes: QKV projections, scores, attn-weights, ctx and out-proj run in
fp8(e4m3) with DoubleRow perf mode (256-deep contraction per pass); FFN
runs in bf16; residuals/normalization in f32. Scale bookkeeping: weights
are pre-scaled by SW (QKV) / SWO (Wo) into fp8 range host-side and the
inverse is folded into the PSUM->SBUF copies; exp() output carries 1/8
(bias -ln8) to stay under fp8-e4m3's max; ctx is normalized by the
softmax denominator via a PE-broadcast 32/den row, and the 1/(32*SWO) is
folded into the out-proj PSUM->SBUF copy.

Softmax needs no max-subtraction (scores ~ N(0,1)); the denominator is a
DoubleRow ones-column matmul over the exp'd score tiles.
"""

import math
from contextlib import ExitStack

import numpy as np
import ml_dtypes

import concourse.bass as bass
import concourse.mybir as mybir
import concourse.tile as tile
from concourse.bass_utils import run_bass_kernel_spmd
from concourse.masks import make_identity

f32 = mybir.dt.float32
f32r = mybir.dt.float32r
bf16 = mybir.dt.bfloat16
fp8 = mybir.dt.float8e4
FP8NP = ml_dtypes.float8_e4m3
BF16NP = ml_dtypes.bfloat16
AF = mybir.ActivationFunctionType
ALU = mybir.AluOpType
DRM = mybir.MatmulPerfMode.DoubleRow
EPS = 1e-5
P = 128
LN8 = math.log(8.0)

SW = 16.0    # fp8 scale for Wq/Wk/Wv
SWO = 64.0   # fp8 scale for Wo
CCT = 32.0   # fp8 scale for normalized ctx

CFG = dict(B=4, S=1024, D=768, H=12, HL=6, DH=768, DF=3072, n_cores=8)


def _split_waits(nc, max_waits=1):
    skip = (
        mybir.InstEventSemaphore,
        mybir.InstCompareAndBranch, mybir.InstIndirectBranch,
        mybir.InstBranchHint,
    )
    for f in nc.m.functions:
        for bb in f.blocks:
            out = []
            for inst in bb.instructions:
                si = inst.sync_info
                if (si is not None and si.on_wait and len(si.on_wait) > max_waits
                        and not isinstance(inst, skip)
                        and getattr(inst, "engine", None) is not None):
                    waits = list(si.on_wait)
                    extra, keep = waits[:-max_waits], waits[-max_waits:]
                    for j, w in enumerate(extra):
                        nop = mybir.InstNoOp(name=f"{inst.name}-wsplit{j}")
                        nop.engine = inst.engine
                        nop.sync_info = mybir.SyncInfo(on_wait=[w], on_update=[])
                        out.append(nop)
                    inst.sync_info = mybir.SyncInfo(
                        on_wait=keep, on_update=list(si.on_update or []))
                out.append(inst)
            bb.instructions = out
    return nc


def _ln_stats(nc, pool, x_ap, d, eps_tile, name):
    fmax = math.gcd(nc.vector.BN_STATS_FMAX, d)
    nsg = d // fmax
    stats = pool.tile([P, nsg, nc.vector.BN_STATS_DIM], f32,
                      tag=f"stats_{name}", name=f"stats_{name}")
    xg = x_ap.rearrange("p (g f) -> p g f", g=nsg)
    for sg in range(nsg):
        nc.vector.bn_stats(out=stats[:, sg, :], in_=xg[:, sg, :])
    mv = pool.tile([P, nc.vector.BN_AGGR_DIM], f32, tag=f"mv_{name}",
                   name=f"mv_{name}")
    nc.vector.bn_aggr(out=mv, in_=stats)
    r = pool.tile([P, 1], f32, tag=f"r_{name}", name=f"r_{name}")
    nc.scalar.activation(out=r, in_=mv[:, 1:2], func=AF.Sqrt, bias=eps_tile,
                         scale=1.0)
    nc.vector.reciprocal(out=r, in_=r)
    return mv[:, 0:1], r


def _build(cfg, use_bv, use_b2, loop=1):
    B, S, D, H, HL, DH, DF = (cfg[k] for k in
                              ("B", "S", "D", "H", "HL", "DH", "DF"))
    nd, ne, nt, nf = D // P, DH // P, S // P, DF // P
    Sq = 512            # own tokens per core
    nj = HL * ne        # out-proj contraction tiles (36)
    sc_exp = float(DH) ** -0.5

    nc = bass.Bass()
    HDD = HL * D * DH
    b8_d = nc.dram_tensor("b8", [3 * HDD + HL * DH * D], fp8,
                          kind="ExternalInput")
    b16_d = nc.dram_tensor("b16", [D * DF + DF * D], bf16,
                           kind="ExternalInput")
    n32 = S * D + Sq * D + 2 * P * HL * ne + P * nf
    b32_d = nc.dram_tensor("b32", [n32], f32, kind="ExternalInput")
    if use_bv:
        bv_d = nc.dram_tensor("bv2", [1, HL, DH], f32r, kind="ExternalInput")
    if use_b2:
        b2_d = nc.dram_tensor("b2r", [1, D], f32r, kind="ExternalInput")
    y_d = nc.dram_tensor("y", [Sq, D], f32, kind="ExternalOutput")

    wq_d = b8_d[0:HDD].rearrange("(h d p e) -> h p d e", h=HL, d=nd, p=P)
    wk_d = b8_d[HDD:2 * HDD].rearrange("(h d p e) -> h p d e", h=HL, d=nd,
                                       p=P)
    wv_d = b8_d[2 * HDD:3 * HDD].rearrange("(h d p e) -> h p d e", h=HL,
                                           d=nd, p=P)
    wo_v = b8_d[3 * HDD:].rearrange("(j p d) -> p j d", j=HL * ne, p=P)
    w1_v = b16_d[0:D * DF].rearrange("(d p f) -> p d f", d=nd, p=P)
    w2_v = b16_d[D * DF:].rearrange("(f p d) -> p f d", f=nf, p=P)
    o = 0
    x_v = b32_d[o:o + S * D].rearrange("(t p d) -> p t d", t=nt, p=P)
    o += S * D
    xres_v = b32_d[o:o + Sq * D].rearrange("(t p d) -> p t d", t=4, p=P)
    o += Sq * D
    bq_v = b32_d[o:o + P * HL * ne].rearrange("(p h e) -> p h e", p=P, h=HL)
    o += P * HL * ne
    bk_v = b32_d[o:o + P * HL * ne].rearrange("(p h e) -> p h e", p=P, h=HL)
    o += P * HL * ne
    b1_v = b32_d[o:o + P * nf].rearrange("(p f) -> p f", p=P)

    groups = [[0, 1], [2, 3], [4, 5], [6, 7]]

    with tile.TileContext(nc) as tc, ExitStack() as ctx:
        singles = ctx.enter_context(tc.tile_pool(name="singles", bufs=1))
        sm = ctx.enter_context(tc.tile_pool(name="sm", bufs=2))
        psA = ctx.enter_context(tc.tile_pool(name="psA", bufs=4, space="PSUM"))
        psT = ctx.enter_context(tc.tile_pool(name="psT", bufs=2, space="PSUM"))
        psPD = ctx.enter_context(tc.tile_pool(name="psPD", bufs=2, space="PSUM"))
        dram = ctx.enter_context(tc.tile_pool(name="dram", bufs=1,
                                              space="DRAM"))

        identb = singles.tile([P, P], bf16)
        make_identity(nc, identb)
        eps_t = singles.tile([P, 1], f32)
        nc.vector.memset(eps_t, EPS)
        mln8 = singles.tile([P, 1], f32)
        nc.vector.memset(mln8, -LN8)
        rsw = singles.tile([P, 1], f32)
        nc.vector.memset(rsw, 1.0 / SW)
        ones_f = singles.tile([P, 2, P], f32)
        nc.vector.memset(ones_f, 1.0 / CCT)
        ones8 = singles.tile([P, 2, P], fp8)
        nc.vector.tensor_copy(ones8, ones_f)
        onesf2 = singles.tile([1, P], f32)
        nc.vector.memset(onesf2, 1.0)
        onesr = singles.tile([1, P], f32r)
        nc.vector.tensor_copy(onesr, onesf2)

        bq_c = singles.tile([P, HL, ne], f32)
        bk_c = singles.tile([P, HL, ne], f32)
        b1_c = singles.tile([P, nf], f32)
        if use_bv:
            bv_r = singles.tile([1, HL, DH], f32r)
            nc.sync.dma_start(out=bv_r, in_=bv_d[0:1])
        if use_b2:
            b2_r = singles.tile([1, D], f32r)
            nc.sync.dma_start(out=b2_r, in_=b2_d[0:1])

        # big weights: tiles here, DMAs deferred into the head loop so the
        # LN1 x-tile DMAs go first in the queue
        wo8t = singles.tile([P, nj, D], fp8)

        # DRAM bounce buffers for the four quarter ReduceScatters
        rsin = [dram.tile([256, D], bf16, name=f"rsin{i}", tag=f"rsin{i}")
                for i in range(4)]
        rsout = [dram.tile([P, D], bf16, name=f"rsout{i}",
                           tag=f"rsout{i}") for i in range(4)]

      # noqa: E999 placeholder
        for _it in range(loop):
            _iter_body(locals())
    return _split_waits(nc)


def _iter_body(env):
    (nc, tc, ctx, use_bv, use_b2, singles, sm, psA, psT, psPD, dram,
     identb, eps_t, mln8, rsw, ones8, onesr, bq_c, bk_c, b1_c, wo8t,
     rsin, rsout, wq_d, wk_d, wv_d, wo_v, w1_v, w2_v, x_v, xres_v,
     bq_v, bk_v, b1_v, y_d, groups, HL, S, D, DH, DF, nd, ne, nt, nf,
     nj, Sq, sc_exp) = (
        env[k] for k in (
            "nc", "tc", "ctx", "use_bv", "use_b2", "singles", "sm", "psA",
            "psT", "psPD", "dram", "identb", "eps_t", "mln8", "rsw",
            "ones8", "onesr", "bq_c", "bk_c", "b1_c", "wo8t", "rsin",
            "rsout", "wq_d", "wk_d", "wv_d", "wo_v", "w1_v", "w2_v",
            "x_v", "xres_v", "bq_v", "bk_v", "b1_v", "y_d", "groups",
            "HL", "S", "D", "DH", "DF", "nd", "ne", "nt", "nf", "nj",
            "Sq", "sc_exp"))
    bv_r = env.get("bv_r")
    b2_r = env.get("b2_r")

    with ExitStack() as itx:
        # ---- phase 1: LN1 + transposes -> h0T8 [D, S] fp8
        h0_pool = itx.enter_context(tc.tile_pool(name="h0_pool", bufs=1))
        h0T8 = h0_pool.tile([P, nd, S], fp8)
        head_stack = itx.enter_context(ExitStack())
        wt_pool = head_stack.enter_context(tc.tile_pool(name="wt", bufs=2))
        hp = head_stack.enter_context(tc.tile_pool(name="hp", bufs=2))
        ctx_pool = head_stack.enter_context(tc.tile_pool(name="ctxp", bufs=1))
        wq_r, wk_r, wv_r = wq_d, wk_d, wv_d

        def _head_w(h):
            wv8t = wt_pool.tile([P, nd, DH], fp8, tag="wv", name="wv8t")
            nc.sync.dma_start(out=wv8t, in_=wv_r[h])
            wq8t = wt_pool.tile([P, nd, DH], fp8, tag="wq", name="wq8t")
            nc.sync.dma_start(out=wq8t, in_=wq_r[h])
            wk8t = wt_pool.tile([P, nd, DH], fp8, tag="wk", name="wk8t")
            nc.sync.dma_start(out=wk8t, in_=wk_r[h])
            return wq8t, wk8t, wv8t

        w_next = _head_w(0)
        nc.sync.dma_start(out=bq_c, in_=bq_v)
        nc.sync.dma_start(out=bk_c, in_=bk_v)
        nc.sync.dma_start(out=b1_c, in_=b1_v)
        with tc.tile_pool(name="ph1", bufs=3) as ph1:
            src_t = x_v
            for st in range(nt):
                xt = ph1.tile([P, D], f32, tag="xt", name="xt")
                nc.sync.dma_start(out=xt, in_=src_t[:, st])
                mu, r = _ln_stats(nc, ph1, xt, D, eps_t, "ln1")
                nmr = ph1.tile([P, 1], f32, tag="nmr", name="nmr")
                nc.vector.tensor_scalar(out=nmr, in0=mu, scalar1=r,
                                        scalar2=-1.0, op0=ALU.mult,
                                        op1=ALU.mult)
                h0 = ph1.tile([P, D], bf16, tag="h0", name="h0")
                nc.scalar.activation(out=h0, in_=xt, func=AF.Identity,
                                     scale=r, bias=nmr)
                for dch in range(nd):
                    ptr = psT.tile([P, P], bf16, tag="tr", name="ptr")
                    nc.tensor.transpose(ptr, h0[:, dch * P:(dch + 1) * P],
                                        identb)
                    if dch % 2 == 0:
                        nc.vector.tensor_copy(
                            h0T8[:, dch, st * P:(st + 1) * P], ptr)
                    else:
                        nc.scalar.activation(
                            out=h0T8[:, dch, st * P:(st + 1) * P], in_=ptr,
                            func=AF.Copy)

        # w1t allocated after ph1 closes so it reuses that SBUF space
        w1p = itx.enter_context(tc.tile_pool(name="w1p", bufs=1, side="right"))
        w1t = w1p.tile([P, nd, DF], bf16)

        # ---- phase 2: per-head QKV + attention; ctx collected for all heads
        ctxAll = ctx_pool.tile([P, nj, S], fp8)
        for h in range(HL):
            wq8t, wk8t, wv8t = w_next
            if h == 2:
                nc.sync.dma_start(out=wo8t, in_=wo_v)
            elif h == 3:
                nc.sync.dma_start(out=w1t, in_=w1_v)
            if h + 1 < HL:
                w_next = _head_w(h + 1)

            vv8 = hp.tile([P, nt, DH], fp8, tag="vv", name="vv8")
            for tt in range(nt):
                pvs = [psA.tile([P, 512], f32, tag="mm", name=f"pv{eh}")
                       for eh in range(2)]
                nmm = nd // 2 + (1 if use_bv else 0)
                for dp in range(nd // 2):
                    for eh in range(2):
                        nc.tensor.matmul(
                            pvs[eh][:, :384],
                            h0T8[:, 2 * dp:2 * dp + 2, tt * P:(tt + 1) * P],
                            wv8t[:, 2 * dp:2 * dp + 2,
                                 eh * 384:eh * 384 + 384],
                            start=(dp == 0), stop=(dp == nmm - 1),
                            perf_mode=DRM)
                if use_bv:
                    for eh in range(2):
                        nc.tensor.matmul(pvs[eh][:, :384], onesr,
                                         bv_r[0:1, h,
                                              eh * 384:eh * 384 + 384],
                                         start=False, stop=True)
                for eh in range(2):
                    nc.scalar.activation(out=vv8[:, tt,
                                                 eh * 384:eh * 384 + 384],
                                         in_=pvs[eh][:, :384], func=AF.Copy,
                                         scale=1.0 / SW)

            qT8 = hp.tile([P, ne, S], fp8, tag="qT", name="qT8")
            kT8 = hp.tile([P, ne, S], fp8, tag="kT", name="kT8")
            for et in range(ne):
                pqs = [psA.tile([P, 512], f32, tag="mm", name=f"pq{qh}")
                       for qh in range(2)]
                for dp in range(nd // 2):
                    for qh in range(2):
                        nc.tensor.matmul(
                            pqs[qh],
                            wq8t[:, 2 * dp:2 * dp + 2, et * P:(et + 1) * P],
                            h0T8[:, 2 * dp:2 * dp + 2, qh * 512:qh * 512 + 512],
                            start=(dp == 0), stop=(dp == nd // 2 - 1),
                            perf_mode=DRM)
                for qh in range(2):
                    nc.scalar.activation(
                        out=qT8[:, et, qh * 512:qh * 512 + 512], in_=pqs[qh],
                        func=AF.Identity, bias=bq_c[:, h, et:et + 1],
                        scale=1.0 / SW)
                pks = [psA.tile([P, 512], f32, tag="mm", name=f"pk{qh}")
                       for qh in range(2)]
                for dp in range(nd // 2):
                    for qh in range(2):
                        nc.tensor.matmul(
                            pks[qh],
                            wk8t[:, 2 * dp:2 * dp + 2, et * P:(et + 1) * P],
                            h0T8[:, 2 * dp:2 * dp + 2, qh * 512:qh * 512 + 512],
                            start=(dp == 0), stop=(dp == nd // 2 - 1),
                            perf_mode=DRM)
                for qh in range(2):
                    nc.vector.tensor_scalar(
                        out=kT8[:, et, qh * 512:qh * 512 + 512], in0=pks[qh],
                        scalar1=rsw, scalar2=bk_c[:, h, et:et + 1],
                        op0=ALU.mult, op1=ALU.add)

            pT8 = hp.tile([P, nt, S], fp8, tag="pT", name="pT8")
            for kb in range(nt):
                pss_ = [psA.tile([P, 512], f32, tag="mm", name=f"ps{qh}")
                        for qh in range(2)]
                for ep in range(ne // 2):
                    for qh in range(2):
                        nc.tensor.matmul(
                            pss_[qh],
                            kT8[:, 2 * ep:2 * ep + 2, kb * P:(kb + 1) * P],
                            qT8[:, 2 * ep:2 * ep + 2, qh * 512:qh * 512 + 512],
                            start=(ep == 0), stop=(ep == ne // 2 - 1),
                            perf_mode=DRM)
                for qh in range(2):
                    nc.scalar.activation(out=pT8[:, kb,
                                                 qh * 512:qh * 512 + 512],
                                         in_=pss_[qh], func=AF.Exp,
                                         scale=sc_exp, bias=mln8)

            # den/32 broadcast to all 128 rows via all-(1/32) DR stationary;
            # one [P,512] reciprocal then yields the 32/den normalizer rows
            pds = [psPD.tile([P, 512], f32, tag="pd", name=f"pd{qh}")
                   for qh in range(2)]
            for tp in range(nt // 2):
                for qh in range(2):
                    nc.tensor.matmul(
                        pds[qh], ones8,
                        pT8[:, 2 * tp:2 * tp + 2, qh * 512:qh * 512 + 512],
                        start=(tp == 0), stop=(tp == nt // 2 - 1),
                        perf_mode=DRM)
            rbs = []
            for qh in range(2):
                rb = sm.tile([P, 512], f32, tag="rb", name="rb")
                nc.vector.reciprocal(out=rb, in_=pds[qh])
                rbs.append(rb)

            for ec in range(ne):
                pcs = [psA.tile([P, 512], f32, tag="mm", name=f"pc{qh}")
                       for qh in range(2)]
                for tp in range(nt // 2):
                    for qh in range(2):
                        nc.tensor.matmul(
                            pcs[qh],
                            vv8[:, 2 * tp:2 * tp + 2, ec * P:(ec + 1) * P],
                            pT8[:, 2 * tp:2 * tp + 2, qh * 512:qh * 512 + 512],
                            start=(tp == 0), stop=(tp == nt // 2 - 1),
                            perf_mode=DRM)
                for qh in range(2):
                    nc.vector.tensor_tensor(
                        out=ctxAll[:, h * ne + ec, qh * 512:qh * 512 + 512],
                        in0=pcs[qh], in1=rbs[qh], op=ALU.mult)

        # ---- phase 3: out-proj (PSUM-accumulated over all heads) + RS
        oscale = 1.0 / (CCT * SWO)
        for quarter in range(4):
            dst = rsin[quarter].rearrange("(t p) d -> p t d", p=P)
            for ttl in range(2):
                tt = quarter * 2 + ttl
                pout = sm.tile([P, D], bf16, tag="pout", name="pout")
                pos = [psA.tile([P, 512], f32, tag="mm", name=f"po{dh}")
                       for dh in range(2)]
                for jp in range(nj // 2):
                    for dh in range(2):
                        nc.tensor.matmul(
                            pos[dh][:, :384],
                            ctxAll[:, 2 * jp:2 * jp + 2, tt * P:(tt + 1) * P],
                            wo8t[:, 2 * jp:2 * jp + 2,
                                 dh * 384:dh * 384 + 384],
                            start=(jp == 0), stop=(jp == nj // 2 - 1),
                            perf_mode=DRM)
                for dh in range(2):
                    nc.scalar.activation(
                        out=pout[:, dh * 384:dh * 384 + 384],
                        in_=pos[dh][:, :384], func=AF.Copy, scale=oscale)
                nc.sync.dma_start(out=dst[:, ttl], in_=pout)
            nc.gpsimd.collective_compute(
                "ReduceScatter", ALU.add, replica_groups=groups,
                ins=[rsin[quarter].opt()], outs=[rsout[quarter].opt()])

        head_stack.close()

        # ---- phase 4: residual + LN2 + FFN, pipelined per 256-token chunk
        with tc.tile_pool(name="ffn", bufs=1) as ffn, \
             tc.tile_pool(name="ph3", bufs=2) as ph3:
            w2t = ffn.tile([P, nf, D], bf16)
            nc.sync.dma_start(out=w2t, in_=w2_v)
            x1 = ffn.tile([P, 4, D], f32)
            xr_t = xres_v
            y_t = y_d.rearrange("(t p) d -> p t d", p=P)
            for chunk in range(2):
                rsc = ph3.tile([P, 2, D], bf16, tag="rsc", name="rsc")
                nc.sync.dma_start(out=rsc[:, 0, :], in_=rsout[chunk * 2][:])
                nc.sync.dma_start(out=rsc[:, 1, :],
                                  in_=rsout[chunk * 2 + 1][:])
                xrc = ph3.tile([P, 2, D], f32, tag="xrc", name="xrc")
                nc.sync.dma_start(out=xrc, in_=xr_t[:, chunk * 2:chunk * 2 + 2])
                nc.vector.tensor_tensor(out=x1[:, chunk * 2:chunk * 2 + 2, :],
                                        in0=rsc, in1=xrc, op=ALU.add)
                h2T = ph3.tile([P, nd, 256], bf16, tag="h2T", name="h2T")
                for tl in range(2):
                    tg = chunk * 2 + tl
                    mu2, r2 = _ln_stats(nc, ph3, x1[:, tg, :], D, eps_t, "ln2")
                    h2 = ph3.tile([P, D], bf16, tag="h2", name="h2")
                    nc.vector.tensor_scalar(out=h2, in0=x1[:, tg, :],
                                            scalar1=mu2, scalar2=r2,
                                            op0=ALU.subtract, op1=ALU.mult)
                    for dch in range(nd):
                        ptr2 = psT.tile([P, P], bf16, tag="tr", name="ptr2")
                        nc.tensor.transpose(ptr2, h2[:, dch * P:(dch + 1) * P],
                                            identb)
                        nc.vector.tensor_copy(
                            h2T[:, dch, tl * P:(tl + 1) * P], ptr2)

                relu1 = ph3.tile([P, nf, 256], bf16, tag="relu1", name="relu1")
                for fp2 in range(nf // 2):
                    pfs = [psA.tile([P, 512], f32, tag="mm", name=f"pf{i}")
                           for i in range(2)]
                    for dch in range(nd):
                        for i in range(2):
                            ft = 2 * fp2 + i
                            nc.tensor.matmul(pfs[i][:, :256],
                                             w1t[:, dch, ft * P:(ft + 1) * P],
                                             h2T[:, dch, :],
                                             start=(dch == 0),
                                             stop=(dch == nd - 1))
                    for i in range(2):
                        ft = 2 * fp2 + i
                        nc.scalar.activation(out=relu1[:, ft, :],
                                             in_=pfs[i][:, :256],
                                             func=AF.Relu,
                                             bias=b1_c[:, ft:ft + 1],
                                             scale=1.0)

                for tl in range(2):
                    tg = chunk * 2 + tl
                    pffs = [psA.tile([P, 512], f32, tag="mm", name=f"pff{dh}")
                            for dh in range(2)]
                    nmm = nf + (1 if use_b2 else 0)
                    for ft in range(nf):
                        for dh in range(2):
                            nc.tensor.matmul(
                                pffs[dh][:, :384],
                                relu1[:, ft, tl * P:(tl + 1) * P],
                                w2t[:, ft, dh * 384:dh * 384 + 384],
                                start=(ft == 0), stop=(ft == nmm - 1))
                    if use_b2:
                        for dh in range(2):
                            nc.tensor.matmul(
                                pffs[dh][:, :384], onesr,
                                b2_r[0:1, dh * 384:dh * 384 + 384],
                                start=False, stop=True)
                    for dh in range(2):
                        d0 = dh * 384
                        yt = ph3.tile([P, 384], f32, tag="yt", name="yt")
                        nc.vector.tensor_tensor(out=yt, in0=pffs[dh][:, :384],
                                                in1=x1[:, tg, d0:d0 + 384],
                                                op=ALU.add)
                        nc.sync.dma_start(out=y_t[:, tg, d0:d0 + 384], in_=yt)


def _fp8(x, scale):
    fmax = float(ml_dtypes.finfo(FP8NP).max) * 0.95
    return np.clip(np.asarray(x, np.float32) * scale, -fmax, fmax).astype(FP8NP)


def _prep_host(cfg, inputs):
    B, S, D, H, HL, DH, DF = (cfg[k] for k in
                              ("B", "S", "D", "H", "HL", "DH", "DF"))
    n_cores = cfg["n_cores"]
    ii = {k: np.asarray(v, dtype=np.float32) for k, v in inputs.items()}
    x = ii["x"]
    g1, be1, g2, be2 = ii["g1"], ii["be1"], ii["g2"], ii["be2"]

    wq_eff = ii["Wq"] * g1[None, :, None]
    wk_eff = ii["Wk"] * g1[None, :, None]
    wv_eff = ii["Wv"] * g1[None, :, None]
    bq_eff = ii["bq"] + np.einsum("d,hde->he", be1, ii["Wq"])
    bk_eff = ii["bk"] + np.einsum("d,hde->he", be1, ii["Wk"])
    bv_eff = ii["bv"] + np.einsum("d,hde->he", be1, ii["Wv"])
    w1_eff = ii["W1"] * g2[:, None]
    b1_eff = ii["b1"] + be2 @ ii["W1"]
    b2_eff = ii["b2"][None]

    use_bv = bool(np.any(bv_eff != 0))
    use_b2 = bool(np.any(b2_eff != 0))

    w1b = w1_eff.astype(BF16NP)
    w2b = ii["W2"].astype(BF16NP)
    wo_h = ii["Wo"].reshape(H, DH, D)

    in_maps = []
    for c in range(n_cores):
        b, hg = c // 2, c % 2
        hs = slice(hg * HL, (hg + 1) * HL)
        own = np.concatenate([
            x[b, q * 256 + hg * 128:q * 256 + hg * 128 + 128]
            for q in range(4)])
        ne_ = DH // P
        bq_arr = bq_eff[hs].reshape(HL, ne_, P).transpose(2, 0, 1)
        bk_arr = bk_eff[hs].reshape(HL, ne_, P).transpose(2, 0, 1)
        b1_arr = b1_eff.reshape(DF // P, P).T
        b8 = np.concatenate([
            _fp8(wq_eff[hs], SW).ravel(), _fp8(wk_eff[hs], SW).ravel(),
            _fp8(wv_eff[hs], SW).ravel(), _fp8(wo_h[hs], SWO).ravel()])
        b16 = np.concatenate([w1b.ravel(), w2b.ravel()])
        b32 = np.concatenate([
            x[b].ravel(), (own + ii["bo"][None, :]).ravel(),
            bq_arr.ravel(), bk_arr.ravel(), b1_arr.ravel()]).astype(np.float32)
        m = dict(b8=b8, b16=b16, b32=b32)
        if use_bv:
            m["bv2"] = np.ascontiguousarray((bv_eff[hs] * SW)[None])
        if use_b2:
            m["b2r"] = np.ascontiguousarray(b2_eff)
        in_maps.append(m)
    return in_maps, use_bv, use_b2


_NC_CACHE = {}


def kernel(**inputs) -> np.ndarray:
    cfg = CFG
    in_maps, use_bv, use_b2 = _prep_host(cfg, inputs)
    key = (use_bv, use_b2)
    if key not in _NC_CACHE:
        _NC_CACHE[key] = _build(cfg, use_bv, use_b2)
    nc = _NC_CACHE[key]
    res = run_bass_kernel_spmd(nc, in_maps, list(range(cfg["n_cores"])))

    B, S, D = cfg["B"], cfg["S"], cfg["D"]
    out = np.empty((B, S, D), np.float32)
    for c in range(cfg["n_cores"]):
        b, hg = c // 2, c % 2
        y = res.results[c]["y"]
        for q in range(4):
            t0 = q * 256 + hg * 128
            out[b, t0:t0 + 128] = y[q * 128:(q + 1) * 128]
    return out



# revision 5
# speedup vs baseline: 1.2480x; 1.2480x over previous
"""Transformer block on 8 Trainium2 NeuronCores — head-sharded fp8 version
with host-side weight folding.

Sharding: batch (4) x head-half (2). Core c owns batch b = c//2 and heads
[hg*6, hg*6+6), hg = c%2, over the FULL 1024-token sequence. Each core runs
LN1 + its 6 heads' attention and the row-shard of the output projection,
producing a partial attention output for all 1024 tokens. The pair's
partials are summed with a per-quarter ReduceScatter (groups
[[0,1],[2,3],[4,5],[6,7]]), leaving each core 4x128 own tokens; each core
then adds its residual slice (x + bo_eff, prepped host-side), runs LN2 +
FFN on its 512 tokens and writes y.

Algebraic folding (host-side, exact):
  scores = (h Wq + bq)(h Wk + bk)^T -> h A h^T + per-key bias, with
  A = Wq Wk^T. The bk / const terms are row-constant and cancel in
  softmax; the bq term is v_t = h_t . (Wk bq), a per-key-token additive
  bias folded into the exp (only emitted when bq_eff != 0).
  ctx @ Wo = (attn @ h) @ (Wv Wo) + bv @ Wo (softmax rows sum to 1), so
  the V projection disappears: Wvo = Wv Wo and bv@Wo folds into bo.
This removes the K and V projections (and their weights) entirely.

Dtypes: A-projection, scores, attn-weights, ctx and out-proj run in
fp8(e4m3) with DoubleRow perf mode (256-deep contraction per pass); FFN
runs in bf16 (fp8 FFN busts the 2e-2 gate); residuals/normalization in
f32. Scale bookkeeping: A is pre-scaled by SA, Wvo by SVO into fp8 range
host-side; Qt is stored at SQ (evac scale SQ/SA); exp() output carries
1/8 (bias -ln8) to stay under fp8-e4m3's max; ctx is normalized by the
softmax denominator via a PE-broadcast 1/CCT ones matmul + reciprocal,
and 1/(CCT*SVO) is folded into the out-proj PSUM->SBUF copy.

PSUM matmuls accumulate into two-bank pair tiles [P, 2, 512] so each
PSUM->SBUF evacuation moves 1024 elements in a single ACT/DVE
instruction (half the per-instruction fixed cost of the per-bank form).
"""

import math
from contextlib import ExitStack

import numpy as np
import ml_dtypes

import concourse.bass as bass
import concourse.mybir as mybir
import concourse.tile as tile
from concourse.bass_utils import run_bass_kernel_spmd
from concourse.masks import make_identity

f32 = mybir.dt.float32
f32r = mybir.dt.float32r
bf16 = mybir.dt.bfloat16
fp8 = mybir.dt.float8e4
FP8NP = ml_dtypes.float8_e4m3
BF16NP = ml_dtypes.bfloat16
AF = mybir.ActivationFunctionType
ALU = mybir.AluOpType
DRM = mybir.MatmulPerfMode.DoubleRow
EPS = 1e-5
P = 128
LN8 = math.log(8.0)

SA = 512.0   # fp8 scale for A = Wq Wk^T
SQ = 16.0    # fp8 scale for stored Qt
SVO = 2048.0  # fp8 scale for Wvo = Wv Wo
CCT = 32.0   # fp8 scale for normalized ctx

CFG = dict(B=4, S=1024, D=768, H=12, HL=6, DH=768, DF=3072, n_cores=8)


def _split_waits(nc, max_waits=1):
    skip = (
        mybir.InstEventSemaphore,
        mybir.InstCompareAndBranch, mybir.InstIndirectBranch,
        mybir.InstBranchHint,
    )
    for f in nc.m.functions:
        for bb in f.blocks:
            out = []
            for inst in bb.instructions:
                si = inst.sync_info
                if (si is not None and si.on_wait and len(si.on_wait) > max_waits
                        and not isinstance(inst, skip)
                        and getattr(inst, "engine", None) is not None):
                    waits = list(si.on_wait)
                    extra, keep = waits[:-max_waits], waits[-max_waits:]
                    for j, w in enumerate(extra):
                        nop = mybir.InstNoOp(name=f"{inst.name}-wsplit{j}")
                        nop.engine = inst.engine
                        nop.sync_info = mybir.SyncInfo(on_wait=[w], on_update=[])
                        out.append(nop)
                    inst.sync_info = mybir.SyncInfo(
                        on_wait=keep, on_update=list(si.on_update or []))
                out.append(inst)
            bb.instructions = out
    return nc


def _ln_stats(nc, pool, x_ap, d, eps_tile, name):
    fmax = math.gcd(nc.vector.BN_STATS_FMAX, d)
    nsg = d // fmax
    stats = pool.tile([P, nsg, nc.vector.BN_STATS_DIM], f32,
                      tag=f"stats_{name}", name=f"stats_{name}")
    xg = x_ap.rearrange("p (g f) -> p g f", g=nsg)
    for sg in range(nsg):
        nc.vector.bn_stats(out=stats[:, sg, :], in_=xg[:, sg, :])
    mv = pool.tile([P, nc.vector.BN_AGGR_DIM], f32, tag=f"mv_{name}",
                   name=f"mv_{name}")
    nc.vector.bn_aggr(out=mv, in_=stats)
    r = pool.tile([P, 1], f32, tag=f"r_{name}", name=f"r_{name}")
    nc.scalar.activation(out=r, in_=mv[:, 1:2], func=AF.Sqrt, bias=eps_tile,
                         scale=1.0)
    nc.vector.reciprocal(out=r, in_=r)
    return mv[:, 0:1], r


def _build(cfg, flags, loop=1):
    use_qb, use_b1, use_b2 = flags
    B, S, D, H, HL, DH, DF = (cfg[k] for k in
                              ("B", "S", "D", "H", "HL", "DH", "DF"))
    nd, ne, nt, nf = D // P, DH // P, S // P, DF // P
    Sq = 512            # own tokens per core
    nj = HL * ne        # out-proj contraction tiles (36)
    sc_exp = float(DH) ** -0.5

    nc = bass.Bass()
    HDD = HL * D * DH
    b8_d = nc.dram_tensor("b8", [2 * HDD], fp8, kind="ExternalInput")
    b16_d = nc.dram_tensor("b16", [D * DF + DF * D], bf16,
                           kind="ExternalInput")
    n32 = S * D + Sq * D + P * nf + (P * nd * HL if use_qb else 0)
    b32_d = nc.dram_tensor("b32", [n32], f32, kind="ExternalInput")
    if use_b2:
        b2_d = nc.dram_tensor("b2r", [1, D], f32r, kind="ExternalInput")
    y_d = nc.dram_tensor("y", [Sq, D], f32, kind="ExternalOutput")

    a_d = b8_d[0:HDD].rearrange("(h d p e) -> h p d e", h=HL, d=nd, p=P)
    wvo_v = b8_d[HDD:].rearrange("(j p d) -> p j d", j=nj, p=P)
    w1_v = b16_d[0:D * DF].rearrange("(d p f) -> p d f", d=nd, p=P)
    w2_v = b16_d[D * DF:].rearrange("(f p d) -> p f d", f=nf, p=P)
    o = 0
    x_v = b32_d[o:o + S * D].rearrange("(t p d) -> p t d", t=nt, p=P)
    o += S * D
    xres_v = b32_d[o:o + Sq * D].rearrange("(t p d) -> p t d", t=4, p=P)
    o += Sq * D
    b1_v = b32_d[o:o + P * nf].rearrange("(p f) -> p f", p=P)
    o += P * nf
    if use_qb:
        wkbq_v = b32_d[o:o + P * nd * HL].rearrange("(p d h) -> p d h", p=P,
                                                    d=nd)

    groups = [[0, 1], [2, 3], [4, 5], [6, 7]]

    with tile.TileContext(nc) as tc, ExitStack() as ctx:
        singles = ctx.enter_context(tc.tile_pool(name="singles", bufs=1))
        sm = ctx.enter_context(tc.tile_pool(name="sm", bufs=2))
        psA = ctx.enter_context(tc.tile_pool(name="psA", bufs=2, space="PSUM"))
        psT = ctx.enter_context(tc.tile_pool(name="psT", bufs=2, space="PSUM"))
        psPD = ctx.enter_context(tc.tile_pool(name="psPD", bufs=1,
                                              space="PSUM"))
        dram = ctx.enter_context(tc.tile_pool(name="dram", bufs=1,
                                              space="DRAM"))

        identb = singles.tile([P, P], bf16)
        make_identity(nc, identb)
        eps_t = singles.tile([P, 1], f32)
        nc.vector.memset(eps_t, EPS)
        mln8 = singles.tile([P, 1], f32)
        nc.vector.memset(mln8, -LN8)
        ones_f = singles.tile([P, 2, P], f32)
        nc.vector.memset(ones_f, 1.0 / CCT)
        ones8 = singles.tile([P, 2, P], fp8)
        nc.vector.tensor_copy(ones8, ones_f)
        if use_b2:
            onesf2 = singles.tile([1, P], f32)
            nc.vector.memset(onesf2, 1.0)
            onesr = singles.tile([1, P], f32r)
            nc.vector.tensor_copy(onesr, onesf2)

        b1_c = singles.tile([P, nf], f32)
        if use_qb:
            wkbq_c = singles.tile([P, nd, HL], f32)
        if use_b2:
            b2_r = singles.tile([1, D], f32r)
            nc.sync.dma_start(out=b2_r, in_=b2_d[0:1])

        # big persistent weights: tiles here, DMAs deferred into the phase
        # loops so the LN1 x-tile DMAs go first in the queue
        wvo8 = singles.tile([P, nj, D], fp8)

        # DRAM bounce buffers for the four quarter ReduceScatters
        rsin = [dram.tile([256, D], bf16, name=f"rsin{i}", tag=f"rsin{i}")
                for i in range(4)]
        rsout = [dram.tile([P, D], bf16, name=f"rsout{i}",
                           tag=f"rsout{i}") for i in range(4)]

        for _it in range(loop):
            _iter_body(locals())
    return _split_waits(nc)


def _iter_body(env):
    (nc, tc, ctx, use_qb, use_b1, use_b2, singles, sm, psA, psT, psPD, dram,
     identb, eps_t, mln8, ones8, b1_c, wvo8, rsin, rsout,
     a_d, wvo_v, w1_v, w2_v, x_v, xres_v, b1_v, y_d, groups,
     HL, S, D, DH, DF, nd, ne, nt, nf, nj, Sq, sc_exp) = (
        env[k] for k in (
            "nc", "tc", "ctx", "use_qb", "use_b1", "use_b2", "singles",
            "sm", "psA", "psT", "psPD", "dram", "identb",
            "eps_t", "mln8", "ones8", "b1_c", "wvo8", "rsin", "rsout",
            "a_d", "wvo_v", "w1_v", "w2_v", "x_v", "xres_v", "b1_v",
            "y_d", "groups", "HL", "S", "D", "DH", "DF", "nd", "ne",
            "nt", "nf", "nj", "Sq", "sc_exp"))
    wkbq_c = env.get("wkbq_c")
    wkbq_v = env.get("wkbq_v")
    b2_r = env.get("b2_r")
    onesr = env.get("onesr")

    with ExitStack() as itx:
        # ---- phase 1: LN1 -> h08 [tok, D] fp8 and h0T8 [D, tok] fp8
        h0_pool = itx.enter_context(tc.tile_pool(name="h0_pool", bufs=1))
        h0T8 = h0_pool.tile([P, nd, S], fp8)
        h08 = h0_pool.tile([P, nt, D], fp8)
        head_stack = itx.enter_context(ExitStack())
        wt_pool = head_stack.enter_context(tc.tile_pool(name="wt", bufs=2))
        hp = head_stack.enter_context(tc.tile_pool(name="hp", bufs=2))
        ctx_pool = head_stack.enter_context(tc.tile_pool(name="ctxp", bufs=1))

        def _head_a(h):
            a8 = wt_pool.tile([P, nd, DH], fp8, tag="a8", name="a8")
            nc.sync.dma_start(out=a8, in_=a_d[h])
            return a8

        a_next = _head_a(0)
        nc.sync.dma_start(out=b1_c, in_=b1_v)
        if use_qb:
            nc.sync.dma_start(out=wkbq_c, in_=wkbq_v)
        with tc.tile_pool(name="ph1", bufs=3) as ph1:
            for st in range(nt):
                xt = ph1.tile([P, D], f32, tag="xt", name="xt")
                nc.sync.dma_start(out=xt, in_=x_v[:, st])
                mu, r = _ln_stats(nc, ph1, xt, D, eps_t, "ln1")
                nmr = ph1.tile([P, 1], f32, tag="nmr", name="nmr")
                nc.vector.tensor_scalar(out=nmr, in0=mu, scalar1=r,
                                        scalar2=-1.0, op0=ALU.mult,
                                        op1=ALU.mult)
                h0 = ph1.tile([P, D], bf16, tag="h0", name="h0")
                nc.scalar.activation(out=h0, in_=xt, func=AF.Identity,
                                     scale=r, bias=nmr)
                nc.gpsimd.tensor_copy(h08[:, st, :], h0)
                for dch in range(nd):
                    ptr = psT.tile([P, P], bf16, tag="tr", name="ptr")
                    nc.tensor.transpose(ptr, h0[:, dch * P:(dch + 1) * P],
                                        identb)
                    if dch % 2 == 0:
                        nc.vector.tensor_copy(
                            h0T8[:, dch, st * P:(st + 1) * P], ptr)
                    else:
                        nc.scalar.activation(
                            out=h0T8[:, dch, st * P:(st + 1) * P], in_=ptr,
                            func=AF.Copy)

        # w1t allocated after ph1 closes so it reuses that SBUF space
        w1p = itx.enter_context(tc.tile_pool(name="w1p", bufs=1, side="right"))
        w1t = w1p.tile([P, nd, DF], bf16)

        # ---- phase 2: per-head Qt / scores / softmax / ctx
        ctxAll = ctx_pool.tile([P, nj, S], fp8)
        for h in range(HL):
            a8 = a_next
            if h == 2:
                nc.sync.dma_start(out=wvo8, in_=wvo_v)
            elif h == 3:
                nc.sync.dma_start(out=w1t, in_=w1_v)
            if h + 1 < HL:
                a_next = _head_a(h + 1)

            if use_qb:
                vb = hp.tile([P, nt], f32, tag="vb", name="vb")
                for tt in range(nt):
                    pv = psT.tile([P, 1], f32, tag="pv", name="pv")
                    for dch in range(nd):
                        nc.tensor.matmul(
                            pv, h0T8[:, dch, tt * P:(tt + 1) * P],
                            wkbq_c[:, dch, h:h + 1],
                            start=(dch == 0), stop=(dch == nd - 1))
                    nc.scalar.activation(out=vb[:, tt:tt + 1], in_=pv,
                                         func=AF.Identity, scale=sc_exp,
                                         bias=mln8)

            qT8 = hp.tile([P, ne, S], fp8, tag="qT", name="qT8")
            for et in range(ne):
                pq = psA.tile([P, 2, 512], f32, tag="mm", name="pq")
                for dp in range(nd // 2):
                    for qh in range(2):
                        nc.tensor.matmul(
                            pq[:, qh, :],
                            a8[:, 2 * dp:2 * dp + 2, et * P:(et + 1) * P],
                            h0T8[:, 2 * dp:2 * dp + 2,
                                 qh * 512:qh * 512 + 512],
                            start=(dp == 0), stop=(dp == nd // 2 - 1),
                            perf_mode=DRM)
                qdst = qT8[:, et, :].rearrange("p (a b) -> p a b", a=2)
                if et % 2 == 0:
                    nc.scalar.activation(out=qdst, in_=pq, func=AF.Copy,
                                         scale=SQ / SA)
                else:
                    nc.vector.tensor_scalar_mul(qdst, pq, SQ / SA)

            pT8 = hp.tile([P, nt, S], fp8, tag="pT", name="pT8")
            for kb in range(nt):
                ps = psA.tile([P, 2, 512], f32, tag="mm", name="ps")
                for ep in range(ne // 2):
                    for qh in range(2):
                        nc.tensor.matmul(
                            ps[:, qh, :],
                            h0T8[:, 2 * ep:2 * ep + 2, kb * P:(kb + 1) * P],
                            qT8[:, 2 * ep:2 * ep + 2,
                                qh * 512:qh * 512 + 512],
                            start=(ep == 0), stop=(ep == ne // 2 - 1),
                            perf_mode=DRM)
                bias_ap = vb[:, kb:kb + 1] if use_qb else mln8
                nc.scalar.activation(
                    out=pT8[:, kb, :].rearrange("p (a b) -> p a b", a=2),
                    in_=ps, func=AF.Exp, scale=sc_exp / SQ, bias=bias_ap)

            # den/CCT broadcast to all 128 rows via all-(1/CCT) DR stationary
            pd = psPD.tile([P, 2, 512], f32, tag="pd", name="pd")
            for tp in range(nt // 2):
                for qh in range(2):
                    nc.tensor.matmul(
                        pd[:, qh, :], ones8,
                        pT8[:, 2 * tp:2 * tp + 2, qh * 512:qh * 512 + 512],
                        start=(tp == 0), stop=(tp == nt // 2 - 1),
                        perf_mode=DRM)
            rb = sm.tile([P, 2, 512], f32, tag="rb", name="rb")
            nc.vector.reciprocal(out=rb, in_=pd)

            for ec in range(ne):
                pc = psA.tile([P, 2, 512], f32, tag="mm", name="pc")
                for tp in range(nt // 2):
                    for qh in range(2):
                        nc.tensor.matmul(
                            pc[:, qh, :],
                            h08[:, 2 * tp:2 * tp + 2, ec * P:(ec + 1) * P],
                            pT8[:, 2 * tp:2 * tp + 2,
                                qh * 512:qh * 512 + 512],
                            start=(tp == 0), stop=(tp == nt // 2 - 1),
                            perf_mode=DRM)
                nc.vector.tensor_tensor(
                    out=ctxAll[:, h * ne + ec, :].rearrange(
                        "p (a b) -> p a b", a=2),
                    in0=pc, in1=rb, op=ALU.mult)

        # ---- phase 3: out-proj (PSUM-accumulated over all heads) + RS
        # w2 prefetch overlaps the out-proj matmuls
        w2p = itx.enter_context(tc.tile_pool(name="w2p", bufs=1, side="right"))
        w2t = w2p.tile([P, nf, D], bf16)
        nc.sync.dma_start(out=w2t, in_=w2_v)

        oscale = 1.0 / (CCT * SVO)
        for quarter in range(4):
            dst = rsin[quarter].rearrange("(t p) d -> p t d", p=P)
            for ttl in range(2):
                tt = quarter * 2 + ttl
                po = psA.tile([P, 2, 512], f32, tag="mm", name="po")
                for jp in range(nj // 2):
                    for dh in range(2):
                        nc.tensor.matmul(
                            po[:, dh, :384],
                            ctxAll[:, 2 * jp:2 * jp + 2,
                                   tt * P:(tt + 1) * P],
                            wvo8[:, 2 * jp:2 * jp + 2,
                                 dh * 384:dh * 384 + 384],
                            start=(jp == 0), stop=(jp == nj // 2 - 1),
                            perf_mode=DRM)
                pout = sm.tile([P, 2, 384], bf16, tag="pout", name="pout")
                nc.scalar.activation(out=pout, in_=po[:, :, :384],
                                     func=AF.Copy, scale=oscale)
                nc.sync.dma_start(
                    out=dst[:, ttl],
                    in_=pout.rearrange("p a b -> p (a b)"))
            nc.gpsimd.collective_compute(
                "ReduceScatter", ALU.add, replica_groups=groups,
                ins=[rsin[quarter].opt()], outs=[rsout[quarter].opt()])

        head_stack.close()

        # ---- phase 4: residual + LN2 + FFN, pipelined per 256-token chunk
        with tc.tile_pool(name="ffn", bufs=1) as ffn, \
             tc.tile_pool(name="ph3", bufs=2) as ph3:
            x1 = ffn.tile([P, 4, D], f32)
            y_t = y_d.rearrange("(t p) d -> p t d", p=P)
            for chunk in range(2):
                rsc = ph3.tile([P, 2, D], bf16, tag="rsc", name="rsc")
                nc.sync.dma_start(out=rsc[:, 0, :], in_=rsout[chunk * 2][:])
                nc.sync.dma_start(out=rsc[:, 1, :],
                                  in_=rsout[chunk * 2 + 1][:])
                xrc = ph3.tile([P, 2, D], f32, tag="xrc", name="xrc")
                nc.sync.dma_start(out=xrc,
                                  in_=xres_v[:, chunk * 2:chunk * 2 + 2])
                nc.vector.tensor_tensor(out=x1[:, chunk * 2:chunk * 2 + 2, :],
                                        in0=rsc, in1=xrc, op=ALU.add)
                h2T = ph3.tile([P, nd, 256], bf16, tag="h2T", name="h2T")
                for tl in range(2):
                    tg = chunk * 2 + tl
                    mu2, r2 = _ln_stats(nc, ph3, x1[:, tg, :], D, eps_t,
                                        "ln2")
                    h2 = ph3.tile([P, D], bf16, tag="h2", name="h2")
                    nc.vector.tensor_scalar(out=h2, in0=x1[:, tg, :],
                                            scalar1=mu2, scalar2=r2,
                                            op0=ALU.subtract, op1=ALU.mult)
                    for dch in range(nd):
                        ptr2 = psT.tile([P, P], bf16, tag="tr", name="ptr2")
                        nc.tensor.transpose(ptr2,
                                            h2[:, dch * P:(dch + 1) * P],
                                            identb)
                        nc.vector.tensor_copy(
                            h2T[:, dch, tl * P:(tl + 1) * P], ptr2)

                relu1 = ph3.tile([P, nf, 256], bf16, tag="relu1",
                                 name="relu1")
                for fp2 in range(nf // 2):
                    pf = psA.tile([P, 2, 512], f32, tag="mm", name="pf")
                    for dch in range(nd):
                        for i in range(2):
                            ft = 2 * fp2 + i
                            nc.tensor.matmul(pf[:, i, :256],
                                             w1t[:, dch, ft * P:(ft + 1) * P],
                                             h2T[:, dch, :],
                                             start=(dch == 0),
                                             stop=(dch == nd - 1))
                    if use_b1:
                        for i in range(2):
                            ft = 2 * fp2 + i
                            nc.scalar.activation(out=relu1[:, ft, :],
                                                 in_=pf[:, i, :256],
                                                 func=AF.Relu,
                                                 bias=b1_c[:, ft:ft + 1],
                                                 scale=1.0)
                    else:
                        nc.scalar.activation(
                            out=relu1[:, 2 * fp2:2 * fp2 + 2, :],
                            in_=pf[:, :, :256], func=AF.Relu, scale=1.0)

                for tl in range(2):
                    tg = chunk * 2 + tl
                    pff = psA.tile([P, 2, 512], f32, tag="mm", name="pff")
                    nmm = nf + (1 if use_b2 else 0)
                    for ft in range(nf):
                        for dh in range(2):
                            nc.tensor.matmul(
                                pff[:, dh, :384],
                                relu1[:, ft, tl * P:(tl + 1) * P],
                                w2t[:, ft, dh * 384:dh * 384 + 384],
                                start=(ft == 0), stop=(ft == nmm - 1))
                    if use_b2:
                        for dh in range(2):
                            nc.tensor.matmul(
                                pff[:, dh, :384], onesr,
                                b2_r[0:1, dh * 384:dh * 384 + 384],
                                start=False, stop=True)
                    yt = ph3.tile([P, 2, 384], f32, tag="yt", name="yt")
                    nc.vector.tensor_tensor(
                        out=yt, in0=pff[:, :, :384],
                        in1=x1[:, tg, :].rearrange("p (a b) -> p a b", a=2),
                        op=ALU.add)
                    nc.sync.dma_start(out=y_t[:, tg, :],
                                      in_=yt.rearrange("p a b -> p (a b)"))


def _fp8(x, scale):
    fmax = 224.0  # stay under TRN e4m3's 240 max (256-448 are NaN on TRN)
    return np.clip(np.asarray(x, np.float32) * scale, -fmax, fmax).astype(FP8NP)


def _prep_host(cfg, inputs):
    B, S, D, H, HL, DH, DF = (cfg[k] for k in
                              ("B", "S", "D", "H", "HL", "DH", "DF"))
    n_cores = cfg["n_cores"]
    ii = {k: np.asarray(v, dtype=np.float32) for k, v in inputs.items()}
    x = ii["x"]
    g1, be1, g2, be2 = ii["g1"], ii["be1"], ii["g2"], ii["be2"]

    wq_eff = ii["Wq"] * g1[None, :, None]
    wk_eff = ii["Wk"] * g1[None, :, None]
    wv_eff = ii["Wv"] * g1[None, :, None]
    bq_eff = ii["bq"] + np.einsum("d,hde->he", be1, ii["Wq"])
    bv_eff = ii["bv"] + np.einsum("d,hde->he", be1, ii["Wv"])
    wo_h = ii["Wo"].reshape(H, DH, D)

    # folded weights
    a_eff = np.einsum("hde,hfe->hdf", wq_eff, wk_eff)      # [H, D, D]
    wvo_eff = np.einsum("hde,hef->hdf", wv_eff, wo_h)      # [H, D, D]
    wkbq_eff = np.einsum("hde,he->hd", wk_eff, bq_eff)     # [H, D]
    bo_eff = ii["bo"] + np.einsum("he,hef->f", bv_eff, wo_h)

    w1_eff = ii["W1"] * g2[:, None]
    b1_eff = ii["b1"] + be2 @ ii["W1"]
    b2_eff = ii["b2"][None]

    use_qb = bool(np.any(bq_eff != 0))
    use_b1 = bool(np.any(b1_eff != 0))
    use_b2 = bool(np.any(b2_eff != 0))
    flags = (use_qb, use_b1, use_b2)

    w1b = w1_eff.astype(BF16NP)
    w2b = ii["W2"].astype(BF16NP)

    nd = D // P
    in_maps = []
    for c in range(n_cores):
        b, hg = c // 2, c % 2
        hs = slice(hg * HL, (hg + 1) * HL)
        own = np.concatenate([
            x[b, q * 256 + hg * 128:q * 256 + hg * 128 + 128]
            for q in range(4)])
        b1_arr = b1_eff.reshape(DF // P, P).T
        b8 = np.concatenate([
            _fp8(a_eff[hs], SA).ravel(), _fp8(wvo_eff[hs], SVO).ravel()])
        b16 = np.concatenate([w1b.ravel(), w2b.ravel()])
        parts = [x[b].ravel(), (own + bo_eff[None, :]).ravel(),
                 b1_arr.ravel()]
        if use_qb:
            # [P, nd, HL]: partition = d % 128, dims (d-chunk, head)
            wkbq_arr = wkbq_eff[hs].T.reshape(nd, P, HL).transpose(1, 0, 2)
            parts.append(wkbq_arr.ravel())
        b32 = np.concatenate(parts).astype(np.float32)
        m = dict(b8=b8, b16=b16, b32=b32)
        if use_b2:
            m["b2r"] = np.ascontiguousarray(b2_eff)
        in_maps.append(m)
    return in_maps, flags


_NC_CACHE = {}


def kernel(**inputs) -> np.ndarray:
    cfg = CFG
    in_maps, flags = _prep_host(cfg, inputs)
    if flags not in _NC_CACHE:
        _NC_CACHE[flags] = _build(cfg, flags)
    nc = _NC_CACHE[flags]
    res = run_bass_kernel_spmd(nc, in_maps, list(range(cfg["n_cores"])))

    B, S, D = cfg["B"], cfg["S"], cfg["D"]
    out = np.empty((B, S, D), np.float32)
    for c in range(cfg["n_cores"]):
        b, hg = c // 2, c % 2
        y = res.results[c]["y"]
        for q in range(4):
            t0 = q * 256 + hg * 128
            out[b, t0:t0 + 128] = y[q * 128:(q + 1) * 128]
    return out


# revision 11
# speedup vs baseline: 1.3373x; 1.0715x over previous
"""Transformer block on 8 Trainium2 NeuronCores — head-sharded fp8 version
with host-side weight folding.

Sharding: batch (4) x head-half (2). Core c owns batch b = c//2 and heads
[hg*6, hg*6+6), hg = c%2, over the FULL 1024-token sequence. Each core runs
LN1 + its 6 heads' attention and the row-shard of the output projection,
producing a partial attention output for all 1024 tokens. The pair's
partials are summed with a per-quarter ReduceScatter (groups
[[0,1],[2,3],[4,5],[6,7]]), leaving each core 4x128 own tokens; each core
then adds its residual slice (x + bo_eff, prepped host-side), runs LN2 +
FFN on its 512 tokens and writes y.

Algebraic folding (host-side, exact):
  scores = (h Wq + bq)(h Wk + bk)^T -> h A h^T + per-key bias, with
  A = Wq Wk^T. The bk / const terms are row-constant and cancel in
  softmax; the bq term is v_t = h_t . (Wk bq), a per-key-token additive
  bias folded into the exp (only emitted when bq_eff != 0).
  ctx @ Wo = (attn @ h) @ (Wv Wo) + bv @ Wo (softmax rows sum to 1), so
  the V projection disappears: Wvo = Wv Wo and bv@Wo folds into bo.
This removes the K and V projections (and their weights) entirely.

Dtypes: A-projection, scores, attn-weights, ctx and out-proj run in
fp8(e4m3) with DoubleRow perf mode (256-deep contraction per pass); FFN
runs in bf16 (fp8 FFN busts the 2e-2 gate); residuals/normalization in
f32. Scale bookkeeping: A is pre-scaled by SA, Wvo by SVO into fp8 range
host-side; Qt is stored at SQ (evac scale SQ/SA); exp() output carries
1/8 (bias -ln8) to stay under fp8-e4m3's max; ctx is normalized by the
softmax denominator via a PE-broadcast 1/CCT ones matmul + reciprocal,
and 1/(CCT*SVO) is folded into the out-proj PSUM->SBUF copy.

PSUM matmuls accumulate into two-bank pair tiles [P, 2, 512] so each
PSUM->SBUF evacuation moves 1024 elements in a single ACT/DVE
instruction (half the per-instruction fixed cost of the per-bank form).
"""

import math
from contextlib import ExitStack

import numpy as np
import ml_dtypes

import concourse.bass as bass
import concourse.mybir as mybir
import concourse.tile as tile
from concourse.bass_utils import run_bass_kernel_spmd
from concourse.masks import make_identity

f32 = mybir.dt.float32
f32r = mybir.dt.float32r
bf16 = mybir.dt.bfloat16
fp8 = mybir.dt.float8e4
FP8NP = ml_dtypes.float8_e4m3
BF16NP = ml_dtypes.bfloat16
AF = mybir.ActivationFunctionType
ALU = mybir.AluOpType
DRM = mybir.MatmulPerfMode.DoubleRow
EPS = 1e-5
P = 128
LN8 = math.log(8.0)

SA = 512.0   # fp8 scale for A = Wq Wk^T
SQ = 16.0    # fp8 scale for stored Qt
SVO = 2048.0  # fp8 scale for Wvo = Wv Wo
CCT = 32.0   # fp8 scale for normalized ctx

CFG = dict(B=4, S=1024, D=768, H=12, HL=6, DH=768, DF=3072, n_cores=8)


def _split_waits(nc, max_waits=1):
    skip = (
        mybir.InstEventSemaphore,
        mybir.InstCompareAndBranch, mybir.InstIndirectBranch,
        mybir.InstBranchHint,
    )
    for f in nc.m.functions:
        for bb in f.blocks:
            out = []
            for inst in bb.instructions:
                si = inst.sync_info
                if (si is not None and si.on_wait and len(si.on_wait) > max_waits
                        and not isinstance(inst, skip)
                        and getattr(inst, "engine", None) is not None):
                    waits = list(si.on_wait)
                    extra, keep = waits[:-max_waits], waits[-max_waits:]
                    for j, w in enumerate(extra):
                        nop = mybir.InstNoOp(name=f"{inst.name}-wsplit{j}")
                        nop.engine = inst.engine
                        nop.sync_info = mybir.SyncInfo(on_wait=[w], on_update=[])
                        out.append(nop)
                    inst.sync_info = mybir.SyncInfo(
                        on_wait=keep, on_update=list(si.on_update or []))
                out.append(inst)
            bb.instructions = out
    return nc


def _ln_stats(nc, pool, x_ap, d, eps_tile, name):
    fmax = math.gcd(nc.vector.BN_STATS_FMAX, d)
    nsg = d // fmax
    stats = pool.tile([P, nsg, nc.vector.BN_STATS_DIM], f32,
                      tag=f"stats_{name}", name=f"stats_{name}")
    xg = x_ap.rearrange("p (g f) -> p g f", g=nsg)
    for sg in range(nsg):
        nc.vector.bn_stats(out=stats[:, sg, :], in_=xg[:, sg, :])
    mv = pool.tile([P, nc.vector.BN_AGGR_DIM], f32, tag=f"mv_{name}",
                   name=f"mv_{name}")
    nc.vector.bn_aggr(out=mv, in_=stats)
    r = pool.tile([P, 1], f32, tag=f"r_{name}", name=f"r_{name}")
    nc.scalar.activation(out=r, in_=mv[:, 1:2], func=AF.Sqrt, bias=eps_tile,
                         scale=1.0)
    nc.vector.reciprocal(out=r, in_=r)
    return mv[:, 0:1], r


def _build(cfg, flags, loop=1):
    use_qb, use_b1, use_b2 = flags
    B, S, D, H, HL, DH, DF = (cfg[k] for k in
                              ("B", "S", "D", "H", "HL", "DH", "DF"))
    nd, ne, nt, nf = D // P, DH // P, S // P, DF // P
    Sq = 512            # own tokens per core
    nj = HL * ne        # out-proj contraction tiles (36)
    sc_exp = float(DH) ** -0.5

    nc = bass.Bass()
    HDD = HL * D * DH
    b8_d = nc.dram_tensor("b8", [2 * HDD], fp8, kind="ExternalInput")
    b16_d = nc.dram_tensor("b16", [D * DF + DF * D + S * D + Sq * D],
                           bf16, kind="ExternalInput")
    n32 = P * nf + (P * nd * HL if use_qb else 0)
    b32_d = nc.dram_tensor("b32", [n32], f32, kind="ExternalInput")
    if use_b2:
        b2_d = nc.dram_tensor("b2r", [1, D], f32r, kind="ExternalInput")
    y_d = nc.dram_tensor("y", [Sq, D], f32, kind="ExternalOutput")

    a_d = b8_d[0:HDD].rearrange("(h d p e) -> h p d e", h=HL, d=nd, p=P)
    wvo_v = b8_d[HDD:].rearrange("(j p d) -> p j d", j=nj, p=P)
    w1_v = b16_d[0:D * DF].rearrange("(d p f) -> p d f", d=nd, p=P)
    w2_v = b16_d[D * DF:2 * D * DF].rearrange("(f p d) -> p f d", f=nf,
                                              p=P)
    o16 = 2 * D * DF
    x_v = b16_d[o16:o16 + S * D].rearrange("(t p d) -> p t d", t=nt, p=P)
    o16 += S * D
    xres_v = b16_d[o16:o16 + Sq * D].rearrange("(t p d) -> p t d", t=4, p=P)
    o = 0
    b1_v = b32_d[o:o + P * nf].rearrange("(p f) -> p f", p=P)
    o += P * nf
    if use_qb:
        wkbq_v = b32_d[o:o + P * nd * HL].rearrange("(p d h) -> p d h", p=P,
                                                    d=nd)

    groups = [[0, 1], [2, 3], [4, 5], [6, 7]]

    with tile.TileContext(nc) as tc, ExitStack() as ctx:
        singles = ctx.enter_context(tc.tile_pool(name="singles", bufs=1))
        sm = ctx.enter_context(tc.tile_pool(name="sm", bufs=2))
        psA = ctx.enter_context(tc.tile_pool(
            name="psA", bufs=(2 if use_qb else 3), space="PSUM"))
        psT = ctx.enter_context(tc.tile_pool(name="psT", bufs=2, space="PSUM"))
        dram = ctx.enter_context(tc.tile_pool(name="dram", bufs=1,
                                              space="DRAM"))

        identb = singles.tile([P, P], bf16)
        make_identity(nc, identb)
        eps_t = singles.tile([P, 1], f32)
        nc.vector.memset(eps_t, EPS)
        mln8 = singles.tile([P, 1], f32)
        nc.vector.memset(mln8, -LN8)
        ones8 = singles.tile([P, 2, P], fp8)
        nc.vector.memset(ones8, 1.0 / CCT)
        if use_b2:
            onesf2 = singles.tile([1, P], f32)
            nc.vector.memset(onesf2, 1.0)
            onesr = singles.tile([1, P], f32r)
            nc.vector.tensor_copy(onesr, onesf2)

        b1_c = singles.tile([P, nf], f32)
        if use_qb:
            wkbq_c = singles.tile([P, nd, HL], f32)
        if use_b2:
            b2_r = singles.tile([1, D], f32r)
            nc.sync.dma_start(out=b2_r, in_=b2_d[0:1])

        # big persistent weights: tiles here, DMAs deferred into the phase
        # loops so the LN1 x-tile DMAs go first in the queue
        wvo8 = singles.tile([P, nj, D], fp8)

        # DRAM bounce buffers for the four quarter ReduceScatters
        rsin = [dram.tile([256, D], bf16, name=f"rsin{i}", tag=f"rsin{i}")
                for i in range(4)]
        rsout = [dram.tile([P, D], bf16, name=f"rsout{i}",
                           tag=f"rsout{i}") for i in range(4)]

        for _it in range(loop):
            _iter_body(locals())
    return _split_waits(nc)


def _iter_body(env):
    (nc, tc, ctx, use_qb, use_b1, use_b2, singles, sm, psA, psT, dram,
     identb, eps_t, mln8, ones8, b1_c, wvo8, rsin, rsout,
     a_d, wvo_v, w1_v, w2_v, x_v, xres_v, b1_v, y_d, groups,
     HL, S, D, DH, DF, nd, ne, nt, nf, nj, Sq, sc_exp) = (
        env[k] for k in (
            "nc", "tc", "ctx", "use_qb", "use_b1", "use_b2", "singles",
            "sm", "psA", "psT", "dram", "identb",
            "eps_t", "mln8", "ones8", "b1_c", "wvo8", "rsin", "rsout",
            "a_d", "wvo_v", "w1_v", "w2_v", "x_v", "xres_v", "b1_v",
            "y_d", "groups", "HL", "S", "D", "DH", "DF", "nd", "ne",
            "nt", "nf", "nj", "Sq", "sc_exp"))
    wkbq_c = env.get("wkbq_c")
    wkbq_v = env.get("wkbq_v")
    b2_r = env.get("b2_r")
    onesr = env.get("onesr")

    with ExitStack() as itx:
        # ---- phase 1: LN1 -> h08 [tok, D] fp8 and h0T8 [D, tok] fp8
        h0_pool = itx.enter_context(tc.tile_pool(name="h0_pool", bufs=1))
        h0T8 = h0_pool.tile([P, nd, S], fp8)
        h08 = h0_pool.tile([P, nt, D], fp8)
        ctx_pool = itx.enter_context(tc.tile_pool(name="ctxp", bufs=1))
        head_stack = itx.enter_context(ExitStack())
        wt_pool = head_stack.enter_context(tc.tile_pool(name="wt", bufs=2))
        hp = head_stack.enter_context(tc.tile_pool(name="hp", bufs=2))

        def _head_a(h):
            a8 = wt_pool.tile([P, nd, DH], fp8, tag="a8", name="a8")
            nc.sync.dma_start(out=a8, in_=a_d[h])
            return a8

        a_next = _head_a(0)
        nc.sync.dma_start(out=b1_c, in_=b1_v)
        if use_qb:
            nc.sync.dma_start(out=wkbq_c, in_=wkbq_v)
        with tc.tile_pool(name="ph1", bufs=3) as ph1:
            for st in range(nt):
                xt = ph1.tile([P, D], bf16, tag="xt", name="xt")
                nc.sync.dma_start(out=xt, in_=x_v[:, st])
                mu, r = _ln_stats(nc, ph1, xt, D, eps_t, "ln1")
                nmr = ph1.tile([P, 1], f32, tag="nmr", name="nmr")
                nc.vector.tensor_scalar(out=nmr, in0=mu, scalar1=r,
                                        scalar2=-1.0, op0=ALU.mult,
                                        op1=ALU.mult)
                h0 = ph1.tile([P, D], bf16, tag="h0", name="h0")
                nc.scalar.activation(out=h0, in_=xt, func=AF.Identity,
                                     scale=r, bias=nmr)
                nc.gpsimd.tensor_copy(h08[:, st, :], h0)
                for dcp in range(nd // 2):
                    ptr = psT.tile([P, 2, P], bf16, tag="tr", name="ptr")
                    for k in range(2):
                        dch = 2 * dcp + k
                        nc.tensor.transpose(ptr[:, k, :],
                                            h0[:, dch * P:(dch + 1) * P],
                                            identb)
                    dst = h0T8[:, 2 * dcp:2 * dcp + 2,
                               st * P:(st + 1) * P]
                    if (st * 3 + dcp) % 2 == 0:
                        nc.vector.tensor_copy(dst, ptr)
                    else:
                        nc.scalar.activation(out=dst, in_=ptr, func=AF.Copy)

        # w1t allocated after ph1 closes so it reuses that SBUF space
        w1p = itx.enter_context(tc.tile_pool(name="w1p", bufs=1, side="right"))
        w1t = w1p.tile([P, nd, DF], bf16)

        # ---- phase 2: per-head Qt / scores / softmax / ctx
        ctxAll = ctx_pool.tile([P, nj, S], fp8)
        for h in range(HL):
            a8 = a_next
            if h == 2:
                nc.sync.dma_start(out=wvo8, in_=wvo_v)
            elif h == 3:
                nc.sync.dma_start(out=w1t, in_=w1_v)
            if h + 1 < HL:
                a_next = _head_a(h + 1)

            if use_qb:
                vb = hp.tile([P, nt], f32, tag="vb", name="vb")
                for tt in range(nt):
                    pv = psT.tile([P, 1], f32, tag="pv", name="pv")
                    for dch in range(nd):
                        nc.tensor.matmul(
                            pv, h0T8[:, dch, tt * P:(tt + 1) * P],
                            wkbq_c[:, dch, h:h + 1],
                            start=(dch == 0), stop=(dch == nd - 1))
                    nc.scalar.activation(out=vb[:, tt:tt + 1], in_=pv,
                                         func=AF.Identity, scale=sc_exp,
                                         bias=mln8)

            qT8 = hp.tile([P, ne, S], fp8, tag="qT", name="qT8")
            for et in range(ne):
                pq = psA.tile([P, 2, 512], f32, tag="mm", name="pq")
                for dp in range(nd // 2):
                    for qh in range(2):
                        nc.tensor.matmul(
                            pq[:, qh, :],
                            a8[:, 2 * dp:2 * dp + 2, et * P:(et + 1) * P],
                            h0T8[:, 2 * dp:2 * dp + 2,
                                 qh * 512:qh * 512 + 512],
                            start=(dp == 0), stop=(dp == nd // 2 - 1),
                            perf_mode=DRM)
                qdst = qT8[:, et, :].rearrange("p (a b) -> p a b", a=2)
                if et % 2 == 0:
                    nc.scalar.activation(out=qdst, in_=pq, func=AF.Copy,
                                         scale=SQ / SA)
                else:
                    nc.vector.tensor_scalar_mul(qdst, pq, SQ / SA)

            pT8 = hp.tile([P, nt, S], fp8, tag="pT", name="pT8")
            for kb in range(nt):
                ps = psA.tile([P, 2, 512], f32, tag="mm", name="ps")
                for ep in range(ne // 2):
                    for qh in range(2):
                        nc.tensor.matmul(
                            ps[:, qh, :],
                            h0T8[:, 2 * ep:2 * ep + 2, kb * P:(kb + 1) * P],
                            qT8[:, 2 * ep:2 * ep + 2,
                                qh * 512:qh * 512 + 512],
                            start=(ep == 0), stop=(ep == ne // 2 - 1),
                            perf_mode=DRM)
                bias_ap = vb[:, kb:kb + 1] if use_qb else mln8
                nc.scalar.activation(
                    out=pT8[:, kb, :].rearrange("p (a b) -> p a b", a=2),
                    in_=ps, func=AF.Exp, scale=sc_exp / SQ, bias=bias_ap)

            # den/CCT broadcast to all 128 rows via all-(1/CCT) DR stationary
            pd = psA.tile([P, 2, 512], f32, tag="mm", name="pd")
            for tp in range(nt // 2):
                for qh in range(2):
                    nc.tensor.matmul(
                        pd[:, qh, :], ones8,
                        pT8[:, 2 * tp:2 * tp + 2, qh * 512:qh * 512 + 512],
                        start=(tp == 0), stop=(tp == nt // 2 - 1),
                        perf_mode=DRM)
            rb = sm.tile([P, 2, 512], f32, tag="rb", name="rb")
            nc.scalar.activation(out=rb, in_=pd, func=AF.Ln, scale=1.0)
            nc.scalar.activation(out=rb, in_=rb, func=AF.Exp, scale=-1.0)

            for ec in range(ne):
                pc = psA.tile([P, 2, 512], f32, tag="mm", name="pc")
                for tp in range(nt // 2):
                    for qh in range(2):
                        nc.tensor.matmul(
                            pc[:, qh, :],
                            h08[:, 2 * tp:2 * tp + 2, ec * P:(ec + 1) * P],
                            pT8[:, 2 * tp:2 * tp + 2,
                                qh * 512:qh * 512 + 512],
                            start=(tp == 0), stop=(tp == nt // 2 - 1),
                            perf_mode=DRM)
                nc.vector.tensor_tensor(
                    out=ctxAll[:, h * ne + ec, :].rearrange(
                        "p (a b) -> p a b", a=2),
                    in0=pc, in1=rb, op=ALU.mult)

        # per-head pools freed before phase 3 so the phase-4 tiles
        # don't alias ctxAll/out-proj state (no false WAR serialization)
        head_stack.close()

        # ---- phase 3: out-proj (PSUM-accumulated over all heads) + RS
        # w2 prefetch overlaps the out-proj matmuls
        w2p = itx.enter_context(tc.tile_pool(name="w2p", bufs=1, side="right"))
        w2t = w2p.tile([P, nf, D], bf16)
        nc.sync.dma_start(out=w2t, in_=w2_v)

        ffn = itx.enter_context(tc.tile_pool(name="ffn", bufs=1))
        ph3 = itx.enter_context(tc.tile_pool(name="ph3", bufs=2))

        oscale = 1.0 / (CCT * SVO)
        for quarter in range(4):
            dst = rsin[quarter].rearrange("(t p) d -> p t d", p=P)
            for ttl in range(2):
                tt = quarter * 2 + ttl
                po = psA.tile([P, 2, 512], f32, tag="mm", name="po")
                for jp in range(nj // 2):
                    for dh in range(2):
                        nc.tensor.matmul(
                            po[:, dh, :384],
                            ctxAll[:, 2 * jp:2 * jp + 2,
                                   tt * P:(tt + 1) * P],
                            wvo8[:, 2 * jp:2 * jp + 2,
                                 dh * 384:dh * 384 + 384],
                            start=(jp == 0), stop=(jp == nj // 2 - 1),
                            perf_mode=DRM)
                pout = sm.tile([P, 2, 384], bf16, tag="pout", name="pout")
                nc.scalar.activation(out=pout, in_=po[:, :, :384],
                                     func=AF.Copy, scale=oscale)
                nc.sync.dma_start(
                    out=dst[:, ttl],
                    in_=pout.rearrange("p a b -> p (a b)"))
            nc.gpsimd.collective_compute(
                "ReduceScatter", ALU.add, replica_groups=groups,
                ins=[rsin[quarter].opt()], outs=[rsout[quarter].opt()])

        # ---- phase 4: residual + LN2 + FFN, pipelined per 256-token chunk
        if True:
            x1 = ffn.tile([P, 4, D], f32)
            y_t = y_d.rearrange("(t p) d -> p t d", p=P)
            for chunk in range(2):
                rsc = ph3.tile([P, 2, D], bf16, tag="rsc", name="rsc")
                nc.sync.dma_start(out=rsc[:, 0, :], in_=rsout[chunk * 2][:])
                nc.sync.dma_start(out=rsc[:, 1, :],
                                  in_=rsout[chunk * 2 + 1][:])
                xrc = ph3.tile([P, 2, D], bf16, tag="xrc", name="xrc")
                nc.sync.dma_start(out=xrc,
                                  in_=xres_v[:, chunk * 2:chunk * 2 + 2])
                nc.vector.tensor_tensor(out=x1[:, chunk * 2:chunk * 2 + 2, :],
                                        in0=rsc, in1=xrc, op=ALU.add)
                h2T = ffn.tile([P, nd, 256], bf16, tag="h2T",
                               name="h2T")
                for tl in range(2):
                    tg = chunk * 2 + tl
                    mu2, r2 = _ln_stats(nc, ph3, x1[:, tg, :], D, eps_t,
                                        "ln2")
                    h2 = ph3.tile([P, D], bf16, tag="h2", name="h2")
                    nc.vector.tensor_scalar(out=h2, in0=x1[:, tg, :],
                                            scalar1=mu2, scalar2=r2,
                                            op0=ALU.subtract, op1=ALU.mult)
                    for dcp in range(nd // 2):
                        ptr2 = psT.tile([P, 2, P], bf16, tag="tr",
                                        name="ptr2")
                        for k in range(2):
                            dch = 2 * dcp + k
                            nc.tensor.transpose(
                                ptr2[:, k, :],
                                h2[:, dch * P:(dch + 1) * P], identb)
                        nc.vector.tensor_copy(
                            h2T[:, 2 * dcp:2 * dcp + 2,
                                tl * P:(tl + 1) * P], ptr2)

                relu1 = ffn.tile([P, nf, 256], bf16, tag="relu1",
                                 name="relu1")
                for fp2 in range(nf // 2):
                    pf = psA.tile([P, 2, 512], f32, tag="mm", name="pf")
                    for dch in range(nd):
                        for i in range(2):
                            ft = 2 * fp2 + i
                            nc.tensor.matmul(pf[:, i, :256],
                                             w1t[:, dch, ft * P:(ft + 1) * P],
                                             h2T[:, dch, :],
                                             start=(dch == 0),
                                             stop=(dch == nd - 1))
                    if use_b1:
                        for i in range(2):
                            ft = 2 * fp2 + i
                            nc.scalar.activation(out=relu1[:, ft, :],
                                                 in_=pf[:, i, :256],
                                                 func=AF.Relu,
                                                 bias=b1_c[:, ft:ft + 1],
                                                 scale=1.0)
                    else:
                        nc.scalar.activation(
                            out=relu1[:, 2 * fp2:2 * fp2 + 2, :],
                            in_=pf[:, :, :256], func=AF.Relu, scale=1.0)

                for tl in range(2):
                    tg = chunk * 2 + tl
                    pff = psA.tile([P, 2, 512], f32, tag="mm", name="pff")
                    nmm = nf + (1 if use_b2 else 0)
                    for ft in range(nf):
                        for dh in range(2):
                            nc.tensor.matmul(
                                pff[:, dh, :384],
                                relu1[:, ft, tl * P:(tl + 1) * P],
                                w2t[:, ft, dh * 384:dh * 384 + 384],
                                start=(ft == 0), stop=(ft == nmm - 1))
                    if use_b2:
                        for dh in range(2):
                            nc.tensor.matmul(
                                pff[:, dh, :384], onesr,
                                b2_r[0:1, dh * 384:dh * 384 + 384],
                                start=False, stop=True)
                    yt = ph3.tile([P, 2, 384], f32, tag="yt", name="yt")
                    nc.vector.tensor_tensor(
                        out=yt, in0=pff[:, :, :384],
                        in1=x1[:, tg, :].rearrange("p (a b) -> p a b", a=2),
                        op=ALU.add)
                    nc.sync.dma_start(out=y_t[:, tg, :],
                                      in_=yt.rearrange("p a b -> p (a b)"))


def _fp8(x, scale):
    fmax = 224.0  # stay under TRN e4m3's 240 max (256-448 are NaN on TRN)
    return np.clip(np.asarray(x, np.float32) * scale, -fmax, fmax).astype(FP8NP)


def _prep_host(cfg, inputs):
    B, S, D, H, HL, DH, DF = (cfg[k] for k in
                              ("B", "S", "D", "H", "HL", "DH", "DF"))
    n_cores = cfg["n_cores"]
    ii = {k: np.asarray(v, dtype=np.float32) for k, v in inputs.items()}
    x = ii["x"]
    g1, be1, g2, be2 = ii["g1"], ii["be1"], ii["g2"], ii["be2"]

    wq_eff = ii["Wq"] * g1[None, :, None]
    wk_eff = ii["Wk"] * g1[None, :, None]
    wv_eff = ii["Wv"] * g1[None, :, None]
    bq_eff = ii["bq"] + np.einsum("d,hde->he", be1, ii["Wq"])
    bv_eff = ii["bv"] + np.einsum("d,hde->he", be1, ii["Wv"])
    wo_h = ii["Wo"].reshape(H, DH, D)

    # folded weights
    a_eff = np.einsum("hde,hfe->hdf", wq_eff, wk_eff)      # [H, D, D]
    wvo_eff = np.einsum("hde,hef->hdf", wv_eff, wo_h)      # [H, D, D]
    wkbq_eff = np.einsum("hde,he->hd", wk_eff, bq_eff)     # [H, D]
    bo_eff = ii["bo"] + np.einsum("he,hef->f", bv_eff, wo_h)

    w1_eff = ii["W1"] * g2[:, None]
    b1_eff = ii["b1"] + be2 @ ii["W1"]
    b2_eff = ii["b2"][None]

    use_qb = bool(np.any(bq_eff != 0))
    use_b1 = bool(np.any(b1_eff != 0))
    use_b2 = bool(np.any(b2_eff != 0))
    flags = (use_qb, use_b1, use_b2)

    w1b = w1_eff.astype(BF16NP)
    w2b = ii["W2"].astype(BF16NP)

    nd = D // P
    in_maps = []
    for c in range(n_cores):
        b, hg = c // 2, c % 2
        hs = slice(hg * HL, (hg + 1) * HL)
        own = np.concatenate([
            x[b, q * 256 + hg * 128:q * 256 + hg * 128 + 128]
            for q in range(4)])
        b1_arr = b1_eff.reshape(DF // P, P).T
        b8 = np.concatenate([
            _fp8(a_eff[hs], SA).ravel(), _fp8(wvo_eff[hs], SVO).ravel()])
        b16 = np.concatenate([
            w1b.ravel(), w2b.ravel(), x[b].astype(BF16NP).ravel(),
            (own + bo_eff[None, :]).astype(BF16NP).ravel()])
        parts = [b1_arr.ravel()]
        if use_qb:
            # [P, nd, HL]: partition = d % 128, dims (d-chunk, head)
            wkbq_arr = wkbq_eff[hs].T.reshape(nd, P, HL).transpose(1, 0, 2)
            parts.append(wkbq_arr.ravel())
        b32 = np.concatenate(parts).astype(np.float32)
        m = dict(b8=b8, b16=b16, b32=b32)
        if use_b2:
            m["b2r"] = np.ascontiguousarray(b2_eff)
        in_maps.append(m)
    return in_maps, flags


_NC_CACHE = {}


def kernel(**inputs) -> np.ndarray:
    cfg = CFG
    in_maps, flags = _prep_host(cfg, inputs)
    if flags not in _NC_CACHE:
        _NC_CACHE[flags] = _build(cfg, flags)
    nc = _NC_CACHE[flags]
    res = run_bass_kernel_spmd(nc, in_maps, list(range(cfg["n_cores"])))

    B, S, D = cfg["B"], cfg["S"], cfg["D"]
    out = np.empty((B, S, D), np.float32)
    for c in range(cfg["n_cores"]):
        b, hg = c // 2, c % 2
        y = res.results[c]["y"]
        for q in range(4):
            t0 = q * 256 + hg * 128
            out[b, t0:t0 + 128] = y[q * 128:(q + 1) * 128]
    return out
